# revision 11
# baseline (speedup 1.0000x reference)
"""MicroHeadAttention Trainium2 kernel v2 (8-core SPMD, data-parallel over
(batch, row-chunk) pairs).

Same decomposition as v1 (each core owns 4 heads: 2 row-pairs p x 2 g),
restructured for HW time:
  - all matmul operands bf16 (host pre-converts weights/x); PSUM stays f32.
  - consolidated DMAs, ordered xt -> wk -> wq -> wv -> (scrambles) -> wo.
  - Q/K projections split by row-pair p (N=256 moving operand) so
    attention for p=0 starts right after the p=0 projections; the p=1
    projections and V/out-proj work are interleaved into attention at
    j5 boundaries as PE filler.
  - Q/K bias copies on DVE (tensor_scalar), freeing ACT for the exp
    stream (the attention-phase pace-setter).
  - attention S tiles are one PSUM tile per 128-wide k block kb holding
    both head-groups [g0 512q | g1 512q]: one exp and (on the diagonal)
    one Pool affine_select (fill=0, post-exp) per kb, and true
    double-buffering within the 8 PSUM banks.
  - V scramble via DRAM round-trip in bf16, issued per row-chunk so
    vsc[p0] is ready before the first ctx matmul.
"""

import numpy as np

import concourse.bass as bass
import concourse.mybir as mybir
from concourse import bacc
from concourse.tile import TileContext
from concourse.bass_utils import run_bass_kernel_spmd

F32 = mybir.dt.float32
BF16 = mybir.dt.bfloat16
DT_MM = BF16
E = 1024
R = 512       # rows per core
RP = 256      # rows per pair
ALU = mybir.AluOpType
ACTF = mybir.ActivationFunctionType

_cache = {}


def _build(loop_n=None, parts="all"):
    nc = bacc.Bacc()
    xT_d = nc.dram_tensor("xT", (E, R), BF16, kind="ExternalInput")
    wq_d = nc.dram_tensor("wqT", (E, E), BF16, kind="ExternalInput")
    wk_d = nc.dram_tensor("wkT", (E, E), BF16, kind="ExternalInput")
    wv_d = nc.dram_tensor("wvT", (E, E), BF16, kind="ExternalInput")
    wo_d = nc.dram_tensor("woTre", (128, 8, E), BF16, kind="ExternalInput")
    bq_d = nc.dram_tensor("bqT", (128, 8), F32, kind="ExternalInput")
    bk_d = nc.dram_tensor("bkT8", (128, 8), F32, kind="ExternalInput")
    bv_d = nc.dram_tensor("bvrow", (1, E), F32, kind="ExternalInput")
    bo_d = nc.dram_tensor("borow", (1, E), F32, kind="ExternalInput")
    out_d = nc.dram_tensor("out", (R, E), BF16, kind="ExternalOutput")

    with TileContext(nc) as tc:
        def body():
            with (
                tc.tile_pool(name="persist", bufs=1) as pp,
                tc.tile_pool(name="pt", bufs=6) as ptp,
                tc.tile_pool(name="misc", bufs=2) as mp,
                tc.tile_pool(name="dram", bufs=1, space="DRAM") as dp,
            ):
                vtmp = dp.tile([2, 2, 2048, 64], DT_MM, tag="vtmp",
                               name="vtmp")
                # ---- persistent tiles ----
                bqT = pp.tile([128, 8], F32, tag="bqT", name="bqT")
                bkT8 = pp.tile([128, 8], F32, tag="bkT8", name="bkT8")
                # q/k scrambled, stored [d-chan, n' = 8j+m]
                qsc = [pp.tile([128, 2048], DT_MM, tag=f"qsc{p}",
                               name=f"qsc{p}") for p in range(2)]
                ksc = [pp.tile([128, 2048], DT_MM, tag=f"ksc{p}",
                               name=f"ksc{p}") for p in range(2)]
                vsc = [[pp.tile([128, 16, 65], DT_MM, tag=f"vsc{p}{g}",
                                name=f"vsc{p}{g}")
                        for g in range(2)] for p in range(2)]
                # ctxP[c, rc, r128, m]: out-proj lhsT reads [:, rc, :, mmv]
                ctxP = [pp.tile([128, 2, 128, 8], DT_MM, tag=f"ctxP{p}",
                                name=f"ctxP{p}")
                        for p in range(2)]
                vnat = [pp.tile([128, 2, E], DT_MM, tag=f"vnat{p}",
                                name=f"vnat{p}")
                        for p in range(2)]
                xt = pp.tile([128, 8, R], DT_MM, tag="xt", name="xt")
                bvr = pp.tile([1, E], F32, tag="bvr", name="bvr")
                bv_bc = pp.tile([128, E], F32, tag="bvbc", name="bvbc")
                bor = pp.tile([1, E], F32, tag="bor", name="bor")
                bo_bc = pp.tile([128, E], F32, tag="bobc", name="bobc")
                wk = pp.tile([128, 8, E], DT_MM, tag="wk", name="wk")
                wq = pp.tile([128, 8, E], DT_MM, tag="wq", name="wq")
                wv = pp.tile([128, 8, E], DT_MM, tag="wv", name="wv")
                wo = pp.tile([128, 8, E], DT_MM, tag="wo", name="wo")

                # small loads off the bulk queue
                nc.scalar.dma_start(bqT[:], bq_d[:])
                nc.scalar.dma_start(bkT8[:], bk_d[:])
                nc.scalar.dma_start(bvr[:], bv_d[:])
                nc.scalar.dma_start(bor[:], bo_d[:])
                nc.gpsimd.partition_broadcast(bv_bc[:], bvr[:])
                nc.gpsimd.partition_broadcast(bo_bc[:], bor[:])
                for p in range(2):
                    for g in range(2):
                        nc.gpsimd.memset(vsc[p][g][:, :, 64], 1.0)

                # bulk DMA in consumption order; xt/wk split so the first
                # K-proj matmul can start after ~5us of loading
                xt_v = xT_d.rearrange("(ko ki) r -> ki ko r", ki=128)
                wk_v = wk_d.rearrange("(ko ki) o -> ki ko o", ki=128)
                nc.scalar.dma_start(xt[:, :, 0:RP], xt_v[:, :, 0:RP])
                nc.scalar.dma_start(xt[:, :, RP:R], xt_v[:, :, RP:R])
                for qtr in range(4):
                    nc.sync.dma_start(wk[:, :, 256 * qtr:256 * (qtr + 1)],
                                      wk_v[:, :, 256 * qtr:256 * (qtr + 1)])
                nc.sync.dma_start(
                    wq[:], wq_d.rearrange("(ko ki) o -> ki ko o", ki=128))
                nc.sync.dma_start(
                    wv[:], wv_d.rearrange("(ko ki) o -> ki ko o", ki=128))

                if parts == "dmaonly":
                    nc.sync.dma_start(wo[:], wo_d[:])
                    nc.sync.dma_start(
                        out_d.rearrange("(a r) o -> r a o", r=128),
                        wv[:, 0:4, :])
                    return

                psa_cm = tc.tile_pool(name="psA", bufs=2, space="PSUM")
                psa = psa_cm.__enter__()
                def psnext():
                    return psa.tile([128, 256], F32, tag="psA", name="psA")

                def qk_proj_half(w_tile, bias_tile, scale, dst, p, t,
                                 use_act=True):
                    # dst[64g+d, 8j+mmv] = scale*(x@W.T) + bias, rows of p
                    ps = psnext()
                    for ki in range(8):
                        nc.tensor.matmul(
                            ps[:], w_tile[:, ki, 128 * t:128 * (t + 1)],
                            xt[:, ki, RP * p:RP * (p + 1)],
                            start=(ki == 0), stop=(ki == 7))
                    g, u = t // 4, t % 4
                    for mh in range(2):
                        mmv = 2 * u + mh
                        dest = dst.rearrange("c (j m) -> c j m", m=8)[
                            64 * g:64 * (g + 1), :, mmv]
                        # the strided interleave writes are slow on every
                        # engine: split DVE/ACT so they run in parallel
                        # (ACT only while it has no exp work)
                        if mh == 1 and use_act:
                            nc.scalar.activation(
                                dest, ps[64:128, :], ACTF.Identity,
                                bias=bias_tile[64:128, t:t + 1], scale=scale)
                        else:
                            nc.vector.tensor_scalar(
                                dest, ps[64 * mh:64 * (mh + 1), :],
                                scale,
                                bias_tile[64 * mh:64 * (mh + 1), t:t + 1],
                                ALU.mult, ALU.add)

                def v_group(rc):
                    # V projection for row-chunk rc + scramble DMAs
                    p, half = rc // 2, rc % 2
                    for oc in range(4):
                        ps = psnext()
                        for ki in range(8):
                            nc.tensor.matmul(
                                ps[:], xt[:, ki, 128 * rc:128 * (rc + 1)],
                                wv[:, ki, 256 * oc:256 * (oc + 1)],
                                start=(ki == 0), stop=(ki == 7))
                        nc.vector.tensor_tensor(
                            vnat[p][:, half, 256 * oc:256 * (oc + 1)],
                            ps[:], bv_bc[:, 256 * oc:256 * (oc + 1)],
                            ALU.add)
                    # vtmp[p, g, 1024 h + 8 r + m, d] = vnat[p][r, h, 512g+64m+d]
                    for g in range(2):
                        src = vnat[p][:, half, 512 * g:512 * (g + 1)] \
                            .rearrange("r (m d) -> r m d", m=8)
                        dst = vtmp[p, g].rearrange(
                            "(h r m) d -> h r m d", h=2, r=128, m=8)[half]
                        nc.sync.dma_start(dst, src)
                    for g in range(2):
                        nc.sync.dma_start(
                            vsc[p][g][:, 8 * half:8 * (half + 1), 0:64],
                            vtmp[p, g][1024 * half:1024 * (half + 1)]
                            .rearrange("(kb pin) d -> pin kb d", pin=128))

                # ---- p=0 projections ----
                for t in range(8):
                    qk_proj_half(wk, bkT8, 0.125, ksc[0], 0, t)
                for t in range(8):
                    qk_proj_half(wq, bqT, 1.0, qsc[0], 0, t)
                v_group(0)
                v_group(1)
                nc.sync.dma_start(wo[:], wo_d[:])

                if parts == "projonly":
                    for t in range(8):
                        qk_proj_half(wk, bkT8, 0.125, ksc[1], 1, t)
                    for t in range(8):
                        qk_proj_half(wq, bqT, 1.0, qsc[1], 1, t)
                    v_group(2)
                    v_group(3)
                    psa_cm.__exit__(None, None, None)
                    nc.sync.dma_start(
                        out_d.rearrange("(a r) o -> r a o", r=128),
                        wv[:, 0:4, :])
                    return  # noqa: B012

                # filler units interleaved into attention at j5 boundaries
                fill_p0 = (
                    [lambda t=t: qk_proj_half(wk, bkT8, 0.125, ksc[1], 1, t,
                                              use_act=False)
                     for t in range(8)]
                    + [lambda t=t: qk_proj_half(wq, bqT, 1.0, qsc[1], 1, t,
                                                use_act=False)
                       for t in range(8)]
                    + [lambda: v_group(2)]
                )
                fill_sched_p0 = [fill_p0[0:3], fill_p0[3:8],
                                 fill_p0[8:13], fill_p0[13:17]]
                fill_sched_p1 = [[lambda: v_group(3)], [], [], []]

                def attention(p, fill_sched, outproj_cb):
                    for j5 in range(4):
                        nkb = 4 * (j5 + 1)
                        ctx_ps = [pcp.tile([65, 512], F32, tag=f"ctxps{g}",
                                           name=f"ctxps{g}")
                                  for g in range(2)]
                        pts = [None] * nkb

                        def s_block(kb):
                            # columns q < off are fully masked: skip them in
                            # S / exp / ctx entirely
                            off = max(0, 128 * (kb - 4 * j5))
                            st = pssp.tile([128, 1024], F32, tag="st",
                                           name="st")
                            for g in range(2):
                                nc.tensor.matmul(
                                    st[:, 512 * g + off:512 * (g + 1)],
                                    ksc[p][64 * g:64 * (g + 1),
                                           128 * kb:128 * (kb + 1)],
                                    qsc[p][64 * g:64 * (g + 1),
                                           512 * j5 + off:512 * (j5 + 1)],
                                    start=True, stop=True)
                            pt = ptp.tile([128, 1024], DT_MM, tag="pt",
                                          name="pt")
                            nc.scalar.activation(
                                pt.rearrange("c (g q) -> c g q", g=2)[
                                    :, :, off:512],
                                st.rearrange("c (g q) -> c g q", g=2)[
                                    :, :, off:512],
                                ACTF.Exp)
                            if kb >= 4 * j5:  # diagonal block
                                # partially-masked window q in [off, off+128):
                                # keep where (q - off) - c >= 0, else 0
                                nc.gpsimd.affine_select(
                                    out=pt.rearrange("c (g q) -> c g q", g=2)[
                                        :, :, off:off + 128],
                                    in_=pt.rearrange("c (g q) -> c g q", g=2)[
                                        :, :, off:off + 128],
                                    compare_op=ALU.is_ge, fill=0.0,
                                    base=0, pattern=[[0, 2], [1, 128]],
                                    channel_multiplier=-1)
                            pts[kb] = (pt, off)

                        def ctx_block(kb):
                            pt, off = pts[kb]
                            for g in range(2):
                                nc.tensor.matmul(
                                    ctx_ps[g][:, off:512],
                                    vsc[p][g][:, kb, :],
                                    pt[:, 512 * g + off:512 * (g + 1)],
                                    start=(kb == 0), stop=(kb == nkb - 1))

                        # software pipeline: S(kb+1) issued before ctx(kb)
                        s_block(0)
                        for kb in range(nkb - 1):
                            s_block(kb + 1)
                            ctx_block(kb)
                        ctx_block(nkb - 1)
                        for g in range(2):
                            rec = mp.tile([1, 512], F32, tag="rec",
                                          name="rec")
                            nc.vector.reciprocal(rec[:], ctx_ps[g][64:65, :])
                            rbc = mp.tile([64, 512], F32, tag="rbc",
                                          name="rbc")
                            nc.gpsimd.partition_broadcast(rbc[:], rec[:])
                            dest = ctxP[p][64 * g:64 * (g + 1), j5 // 2,
                                           64 * (j5 % 2):64 * (j5 % 2) + 64, :]
                            nc.vector.tensor_tensor(
                                dest.rearrange("c r m -> c (r m)"),
                                ctx_ps[g][0:64, :], rbc[:], ALU.mult)
                        for f in fill_sched[j5]:
                            f()
                        if outproj_cb is not None:
                            outproj_cb(j5)

                def outproj_rc(p, rc):
                    # out-proj for one 128-row chunk, 4 x 256-wide psum
                    # accumulators (reuses the psA pool's single bank)
                    for oh in range(2):
                        outsb = mp.tile([128, 512], BF16, tag="outsb",
                                        name="outsb")
                        for oq in range(2):
                            oc = 2 * oh + oq
                            ps = psnext()
                            for mmv in range(8):
                                nc.tensor.matmul(
                                    ps[:], ctxP[p][:, rc, :, mmv],
                                    wo[:, mmv, 256 * oc:256 * (oc + 1)],
                                    start=(mmv == 0), stop=(mmv == 7))
                            nc.vector.tensor_tensor(
                                outsb[:, 256 * oq:256 * (oq + 1)], ps[:],
                                bo_bc[:, 256 * oc:256 * (oc + 1)], ALU.add)
                        nc.sync.dma_start(
                            out_d[RP * p + 128 * rc:RP * p + 128 * (rc + 1),
                                  512 * oh:512 * (oh + 1)],
                            outsb[:])

                with tc.tile_pool(name="psS", bufs=2, space="PSUM") as pssp, \
                     tc.tile_pool(name="psctx", bufs=1, space="PSUM") as pcp:
                    if parts == "noout":
                        attention(0, fill_sched_p0, None)
                        attention(1, fill_sched_p1, None)
                    else:
                        def op_cb0(j5):
                            if j5 == 2:
                                outproj_rc(0, 0)
                        def op_cb1(j5):
                            if j5 == 1:
                                outproj_rc(0, 1)
                            elif j5 == 2:
                                outproj_rc(1, 0)
                        attention(0, fill_sched_p0, op_cb0)
                        attention(1, fill_sched_p1, op_cb1)
                        outproj_rc(1, 1)
                psa_cm.__exit__(None, None, None)
                if parts == "noout":
                    nc.sync.dma_start(
                        out_d.rearrange("(a r) o -> r a o", r=128),
                        wv[:, 0:4, :])

        if loop_n is None:
            body()
        else:
            with tc.For_i(0, loop_n, 1, hint_engines=(
                    mybir.EngineType.PE, mybir.EngineType.Activation,
                    mybir.EngineType.DVE, mybir.EngineType.SP,
                    mybir.EngineType.Pool)):
                body()
    nc.compile()
    return nc


def _get_nc(loop_n=None, parts="all"):
    key = ("nc", loop_n, parts)
    if key not in _cache:
        _cache[key] = _build(loop_n, parts)
    return _cache[key]


def _bf16(a):
    import ml_dtypes
    return np.ascontiguousarray(a.astype(ml_dtypes.bfloat16))


def pack_inputs(x, Wq, bq, Wk, bk, Wv, bv, Wo, bo):
    x = np.asarray(x, np.float32)
    WqT = _bf16(np.asarray(Wq, np.float32).T)
    WkT = _bf16(np.asarray(Wk, np.float32).T)
    WvT = _bf16(np.asarray(Wv, np.float32).T)
    # woTre[64g + d, m, o] = Wo[o, 512g + 64m + d]
    WoTre = _bf16(
        np.asarray(Wo, np.float32).T.reshape(2, 8, 64, E).transpose(0, 2, 1, 3)
        .reshape(128, 8, E))
    bqT = np.ascontiguousarray(np.asarray(bq, np.float32).reshape(8, 128).T)
    bkT8 = np.ascontiguousarray(
        (np.asarray(bk, np.float32) / 8.0).reshape(8, 128).T)
    bvrow = np.asarray(bv, np.float32).reshape(1, E)
    borow = np.asarray(bo, np.float32).reshape(1, E)

    in_maps = []
    for c in range(8):
        xTs = np.empty((E, R), np.float32)
        for p in range(2):
            h = 2 * c + p
            b_, mp_ = divmod(h, 8)
            xTs[:, RP * p:RP * (p + 1)] = x[b_, RP * mp_:RP * (mp_ + 1), :].T
        in_maps.append({
            "xT": _bf16(xTs), "wqT": WqT, "wkT": WkT,
            "wvT": WvT, "woTre": WoTre, "bqT": bqT, "bkT8": bkT8,
            "bvrow": bvrow, "borow": borow,
        })
    return in_maps


def kernel(x, Wq, bq, Wk, bk, Wv, bv, Wo, bo):
    in_maps = pack_inputs(x, Wq, bq, Wk, bk, Wv, bv, Wo, bo)
    nc = _get_nc()
    res = run_bass_kernel_spmd(nc, in_maps, core_ids=list(range(8)))
    out = np.empty((2, 2048, E), np.float32)
    for c in range(8):
        o = np.asarray(res.results[c]["out"], dtype=np.float32)
        for p in range(2):
            h = 2 * c + p
            b_, mp_ = divmod(h, 8)
            out[b_, RP * mp_:RP * (mp_ + 1), :] = o[RP * p:RP * (p + 1), :]
    return out


# revision 19
# speedup vs baseline: 1.2279x; 1.2279x over previous
"""MicroHeadAttention Trainium2 kernel v2 (8-core SPMD, data-parallel over
(batch, row-chunk) pairs).

Same decomposition as v1 (each core owns 4 heads: 2 row-pairs p x 2 g),
restructured for HW time:
  - all matmul operands bf16 (host pre-converts weights/x); PSUM stays f32.
  - consolidated DMAs, ordered xt -> wk -> wq -> wv -> (scrambles) -> wo.
  - Q/K projections split by row-pair p (N=256 moving operand) so
    attention for p=0 starts right after the p=0 projections; the p=1
    projections and V/out-proj work are interleaved into attention at
    j5 boundaries as PE filler.
  - Q/K bias copies on DVE (tensor_scalar), freeing ACT for the exp
    stream (the attention-phase pace-setter).
  - attention S tiles are one PSUM tile per 128-wide k block kb holding
    both head-groups [g0 512q | g1 512q]: one exp and (on the diagonal)
    one Pool affine_select (fill=0, post-exp) per kb, and true
    double-buffering within the 8 PSUM banks.
  - V scramble via DRAM round-trip in bf16, issued per row-chunk so
    vsc[p0] is ready before the first ctx matmul.
"""

import numpy as np

import concourse.bass as bass
import concourse.mybir as mybir
from concourse import bacc
from concourse.tile import TileContext
from concourse.bass_utils import run_bass_kernel_spmd

F32 = mybir.dt.float32
BF16 = mybir.dt.bfloat16
DT_MM = BF16
E = 1024
R = 512       # rows per core
RP = 256      # rows per pair
ALU = mybir.AluOpType
ACTF = mybir.ActivationFunctionType

_cache = {}


def _build(loop_n=None, parts="all"):
    nc = bacc.Bacc()
    xT_d = nc.dram_tensor("xT", (E, R), BF16, kind="ExternalInput")
    wq_d = nc.dram_tensor("wqT", (E, E), BF16, kind="ExternalInput")
    wk_d = nc.dram_tensor("wkT", (E, E), BF16, kind="ExternalInput")
    wv_d = nc.dram_tensor("wvT", (E, E), BF16, kind="ExternalInput")
    wo_d = nc.dram_tensor("woTre", (128, 8, E), BF16, kind="ExternalInput")
    bq_d = nc.dram_tensor("bqT", (128, 8), F32, kind="ExternalInput")
    bk_d = nc.dram_tensor("bkT8", (128, 8), F32, kind="ExternalInput")
    bv_d = nc.dram_tensor("bvrow", (1, E), F32, kind="ExternalInput")
    bo_d = nc.dram_tensor("borow", (1, E), F32, kind="ExternalInput")
    out_d = nc.dram_tensor("out", (R, E), BF16, kind="ExternalOutput")

    with TileContext(nc) as tc:
        def body():
            with (
                tc.tile_pool(name="persist", bufs=1) as pp,
                tc.tile_pool(name="pt", bufs=8) as ptp,
                tc.tile_pool(name="misc", bufs=2) as mp,
                tc.tile_pool(name="dram", bufs=1, space="DRAM") as dp,
            ):
                vtmp = dp.tile([2, 2, 2048, 64], DT_MM, tag="vtmp",
                               name="vtmp")
                # ---- persistent tiles ----
                bqT = pp.tile([128, 8], F32, tag="bqT", name="bqT")
                bkT8 = pp.tile([128, 8], F32, tag="bkT8", name="bkT8")
                # q/k scrambled, stored [d-chan, n' = 8j+m]
                qsc = [pp.tile([128, 2048], DT_MM, tag=f"qsc{p}",
                               name=f"qsc{p}") for p in range(2)]
                ksc = [pp.tile([128, 2048], DT_MM, tag=f"ksc{p}",
                               name=f"ksc{p}") for p in range(2)]
                vsc = [[pp.tile([128, 16, 65], DT_MM, tag=f"vsc{p}{g}",
                                name=f"vsc{p}{g}")
                        for g in range(2)] for p in range(2)]
                # ctxP[c, rc, r128, m]: out-proj lhsT reads [:, rc, :, mmv]
                ctxP = [pp.tile([128, 2, 128, 8], DT_MM, tag=f"ctxP{p}",
                                name=f"ctxP{p}")
                        for p in range(2)]
                vnat = [pp.tile([128, 2, E], DT_MM, tag=f"vnat{p}",
                                name=f"vnat{p}")
                        for p in range(2)]
                xt = pp.tile([128, 8, R], DT_MM, tag="xt", name="xt")
                bvr = pp.tile([1, E], F32, tag="bvr", name="bvr")
                bv_bc = pp.tile([128, E], F32, tag="bvbc", name="bvbc")
                bor = pp.tile([1, E], F32, tag="bor", name="bor")
                bo_bc = pp.tile([128, E], F32, tag="bobc", name="bobc")
                wk = pp.tile([128, 8, E], DT_MM, tag="wk", name="wk")
                wq = pp.tile([128, 8, E], DT_MM, tag="wq", name="wq")
                wv = pp.tile([128, 8, E], DT_MM, tag="wv", name="wv")
                wo = pp.tile([128, 8, E], DT_MM, tag="wo", name="wo")

                # bulk DMA in consumption order; xt/wk split so the
                # first K-proj matmul can start after ~6us of loading.
                # xt leads the scalar queue (biases are only needed at the
                # first PSUM->SBUF copy, ~12us in).
                xt_v = xT_d.rearrange("(ko ki) r -> ki ko r", ki=128)
                wk_v = wk_d.rearrange("(ko ki) o -> ki ko o", ki=128)
                nc.scalar.dma_start(xt[:, :, 0:RP], xt_v[:, :, 0:RP])
                nc.scalar.dma_start(xt[:, :, RP:R], xt_v[:, :, RP:R])
                for qtr in range(4):
                    nc.sync.dma_start(wk[:, :, 256 * qtr:256 * (qtr + 1)],
                                      wk_v[:, :, 256 * qtr:256 * (qtr + 1)])
                nc.scalar.dma_start(bqT[:], bq_d[:])
                nc.scalar.dma_start(bkT8[:], bk_d[:])
                nc.scalar.dma_start(bvr[:], bv_d[:])
                nc.scalar.dma_start(bor[:], bo_d[:])
                nc.gpsimd.partition_broadcast(bv_bc[:], bvr[:])
                nc.gpsimd.partition_broadcast(bo_bc[:], bor[:])
                for p in range(2):
                    for g in range(2):
                        nc.gpsimd.memset(vsc[p][g][:, :, 64], 1.0)
                # diagonal-window multiplicative mask: the partially-masked
                # 128 columns have the same relative pattern for every
                # diagonal k block: keep where qrel - c >= 0
                maskw = pp.tile([128, 2, 128], DT_MM, tag="maskw",
                                name="maskw")
                nc.gpsimd.memset(maskw[:], 1.0)
                nc.gpsimd.affine_select(
                    out=maskw[:], in_=maskw[:], compare_op=ALU.is_ge,
                    fill=0.0, base=0, pattern=[[0, 2], [1, 128]],
                    channel_multiplier=-1)
                nc.sync.dma_start(
                    wq[:], wq_d.rearrange("(ko ki) o -> ki ko o", ki=128))
                nc.sync.dma_start(
                    wv[:], wv_d.rearrange("(ko ki) o -> ki ko o", ki=128))

                if parts == "dmaonly":
                    nc.sync.dma_start(wo[:], wo_d[:])
                    nc.sync.dma_start(
                        out_d.rearrange("(a r) o -> r a o", r=128),
                        wv[:, 0:4, :])
                    return

                psa_cm = tc.tile_pool(name="psA", bufs=2, space="PSUM")
                psa = psa_cm.__enter__()
                # proj-phase-only deep pool: before the attention pools
                # open, 6 PSUM banks are free, so the p0 projections get a
                # 4-deep accumulator ring instead of psA's 2-deep one
                psb_cm = tc.tile_pool(name="psB", bufs=4, space="PSUM")
                psb = psb_cm.__enter__()
                pools = {"cur": psb}

                def psnext():
                    pl = pools["cur"]
                    return pl.tile([128, 256], F32, tag="psA", name="psA")

                def qk_proj_half(w_tile, bias_tile, scale, dst, p, t,
                                 use_act=True):
                    # dst[64g+d, 8j+mmv] = scale*(x@W.T) + bias, rows of p
                    ps = psnext()
                    for ki in range(8):
                        nc.tensor.matmul(
                            ps[:], w_tile[:, ki, 128 * t:128 * (t + 1)],
                            xt[:, ki, RP * p:RP * (p + 1)],
                            start=(ki == 0), stop=(ki == 7))
                    g, u = t // 4, t % 4
                    for mh in range(2):
                        mmv = 2 * u + mh
                        dest = dst.rearrange("c (j m) -> c j m", m=8)[
                            64 * g:64 * (g + 1), :, mmv]
                        # the strided interleave writes are slow on every
                        # engine: split DVE/ACT so they run in parallel
                        # (ACT only while it has no exp work)
                        if mh == 1 and use_act:
                            nc.scalar.activation(
                                dest, ps[64:128, :], ACTF.Identity,
                                bias=bias_tile[64:128, t:t + 1], scale=scale)
                        else:
                            nc.vector.tensor_scalar(
                                dest, ps[64 * mh:64 * (mh + 1), :],
                                scale,
                                bias_tile[64 * mh:64 * (mh + 1), t:t + 1],
                                ALU.mult, ALU.add)

                def v_group(rc):
                    # V projection for row-chunk rc + scramble DMAs
                    p, half = rc // 2, rc % 2
                    for oc in range(4):
                        ps = psnext()
                        for ki in range(8):
                            nc.tensor.matmul(
                                ps[:], xt[:, ki, 128 * rc:128 * (rc + 1)],
                                wv[:, ki, 256 * oc:256 * (oc + 1)],
                                start=(ki == 0), stop=(ki == 7))
                        nc.vector.tensor_tensor(
                            vnat[p][:, half, 256 * oc:256 * (oc + 1)],
                            ps[:], bv_bc[:, 256 * oc:256 * (oc + 1)],
                            ALU.add)
                    # vtmp[p, g, 1024 h + 8 r + m, d] = vnat[p][r, h, 512g+64m+d]
                    for g in range(2):
                        src = vnat[p][:, half, 512 * g:512 * (g + 1)] \
                            .rearrange("r (m d) -> r m d", m=8)
                        dst = vtmp[p, g].rearrange(
                            "(h r m) d -> h r m d", h=2, r=128, m=8)[half]
                        nc.sync.dma_start(dst, src)
                    for g in range(2):
                        nc.sync.dma_start(
                            vsc[p][g][:, 8 * half:8 * (half + 1), 0:64],
                            vtmp[p, g][1024 * half:1024 * (half + 1)]
                            .rearrange("(kb pin) d -> pin kb d", pin=128))

                # ---- p=0 projections ----
                for t in range(8):
                    qk_proj_half(wk, bkT8, 0.125, ksc[0], 0, t)
                for t in range(8):
                    qk_proj_half(wq, bqT, 1.0, qsc[0], 0, t)
                v_group(0)
                v_group(1)
                nc.sync.dma_start(wo[:], wo_d[:])

                if parts == "projonly":
                    for t in range(8):
                        qk_proj_half(wk, bkT8, 0.125, ksc[1], 1, t)
                    for t in range(8):
                        qk_proj_half(wq, bqT, 1.0, qsc[1], 1, t)
                    v_group(2)
                    v_group(3)
                    psb_cm.__exit__(None, None, None)
                    psa_cm.__exit__(None, None, None)
                    nc.sync.dma_start(
                        out_d.rearrange("(a r) o -> r a o", r=128),
                        wv[:, 0:4, :])
                    return  # noqa: B012

                # filler units interleaved into attention at j5 boundaries
                fill_p0 = (
                    [lambda t=t: qk_proj_half(wk, bkT8, 0.125, ksc[1], 1, t,
                                              use_act=False)
                     for t in range(8)]
                    + [lambda: v_group(2)]
                    + [lambda t=t: qk_proj_half(wq, bqT, 1.0, qsc[1], 1, t,
                                                use_act=False)
                       for t in range(8)]
                )
                fill_sched_p0 = [fill_p0[0:6], fill_p0[6:12],
                                 fill_p0[12:16], fill_p0[16:17]]
                fill_sched_p1 = [[lambda: v_group(3)], [], [], []]

                def attention(p, fill_sched, outproj_cb):
                    # flattened (j5, kb) stream, software-pipelined across
                    # j5 blocks: S(i+1) issues before ctx(i) so the per-
                    # block exp-drain bubble overlaps the next block's S.
                    flat = [(j5, kb) for j5 in range(4)
                            for kb in range(4 * (j5 + 1))]
                    ctx_ps_by_j5 = {}
                    pts = {}

                    def get_ctx_ps(j5):
                        if j5 not in ctx_ps_by_j5:
                            ctx_ps_by_j5[j5] = [
                                pcp.tile([65, 512], F32, tag=f"ctxps{g}",
                                         name=f"ctxps{g}")
                                for g in range(2)]
                        return ctx_ps_by_j5[j5]

                    def s_block(j5, kb):
                            # columns q < off are fully masked: skip them in
                            # S / exp / ctx entirely
                            off = max(0, 128 * (kb - 4 * j5))
                            st = pssp.tile([128, 1024], F32, tag="st",
                                           name="st")
                            for g in range(2):
                                nc.tensor.matmul(
                                    st[:, 512 * g + off:512 * (g + 1)],
                                    ksc[p][64 * g:64 * (g + 1),
                                           128 * kb:128 * (kb + 1)],
                                    qsc[p][64 * g:64 * (g + 1),
                                           512 * j5 + off:512 * (j5 + 1)],
                                    start=True, stop=True)
                            pt = ptp.tile([128, 1024], DT_MM, tag="pt",
                                          name="pt")
                            nc.scalar.activation(
                                pt.rearrange("c (g q) -> c g q", g=2)[
                                    :, :, off:512],
                                st.rearrange("c (g q) -> c g q", g=2)[
                                    :, :, off:512],
                                ACTF.Exp)
                            if kb >= 4 * j5:  # diagonal block
                                # partially-masked window q in [off, off+128):
                                # multiply by the precomputed 0/1 pattern
                                # (DVE bf16, faster than a Pool affine_select
                                # and a shorter exp->mask->ctx chain)
                                win = pt.rearrange("c (g q) -> c g q", g=2)[
                                    :, :, off:off + 128]
                                nc.vector.tensor_tensor(
                                    win, win, maskw[:], ALU.mult)
                            pts[(j5, kb)] = (pt, off)

                    def ctx_block(j5, kb):
                            pt, off = pts.pop((j5, kb))
                            ctx_ps = get_ctx_ps(j5)
                            nkb = 4 * (j5 + 1)
                            for g in range(2):
                                nc.tensor.matmul(
                                    ctx_ps[g][:, off:512],
                                    vsc[p][g][:, kb, :],
                                    pt[:, 512 * g + off:512 * (g + 1)],
                                    start=(kb == 0), stop=(kb == nkb - 1))

                    def divide(j5):
                        ctx_ps = ctx_ps_by_j5.pop(j5)
                        for g in range(2):
                            rec = mp.tile([1, 512], F32, tag="rec",
                                          name="rec")
                            nc.vector.reciprocal(rec[:], ctx_ps[g][64:65, :])
                            rbc = mp.tile([64, 512], F32, tag="rbc",
                                          name="rbc")
                            nc.gpsimd.partition_broadcast(rbc[:], rec[:])
                            dest = ctxP[p][64 * g:64 * (g + 1), j5 // 2,
                                           64 * (j5 % 2):64 * (j5 % 2) + 64, :]
                            nc.vector.tensor_tensor(
                                dest.rearrange("c r m -> c (r m)"),
                                ctx_ps[g][0:64, :], rbc[:], ALU.mult)
                        for f in fill_sched[j5]:
                            f()
                        if outproj_cb is not None:
                            outproj_cb(j5)

                    # driver: S one step ahead of ctx, across j5 blocks
                    s_block(*flat[0])
                    for i, (j5, kb) in enumerate(flat):
                        if i + 1 < len(flat):
                            s_block(*flat[i + 1])
                        ctx_block(j5, kb)
                        if kb == 4 * (j5 + 1) - 1:
                            divide(j5)

                def outproj_rc(p, rc):
                    # out-proj for one 128-row chunk, 4 x 256-wide psum
                    # accumulators (reuses the psA pool's single bank);
                    # per-quarter bias-add + store so the post-PE drain
                    # chain after the last matmul stays short
                    for oc in range(4):
                        ps = psnext()
                        for mmv in range(8):
                            nc.tensor.matmul(
                                ps[:], ctxP[p][:, rc, :, mmv],
                                wo[:, mmv, 256 * oc:256 * (oc + 1)],
                                start=(mmv == 0), stop=(mmv == 7))
                        outsb = mp.tile([128, 256], BF16, tag="outsb",
                                        name="outsb")
                        nc.vector.tensor_tensor(
                            outsb[:], ps[:],
                            bo_bc[:, 256 * oc:256 * (oc + 1)], ALU.add)
                        nc.sync.dma_start(
                            out_d[RP * p + 128 * rc:RP * p + 128 * (rc + 1),
                                  256 * oc:256 * (oc + 1)],
                            outsb[:])

                psb_cm.__exit__(None, None, None)
                pools["cur"] = psa
                with tc.tile_pool(name="psS", bufs=2, space="PSUM") as pssp, \
                     tc.tile_pool(name="psctx", bufs=1, space="PSUM") as pcp:
                    if parts == "noout":
                        attention(0, fill_sched_p0, None)
                        attention(1, fill_sched_p1, None)
                    else:
                        def op_cb0(j5):
                            if j5 == 2:
                                outproj_rc(0, 0)
                        def op_cb1(j5):
                            if j5 == 1:
                                outproj_rc(0, 1)
                            elif j5 == 2:
                                outproj_rc(1, 0)
                        attention(0, fill_sched_p0, op_cb0)
                        attention(1, fill_sched_p1, op_cb1)
                        outproj_rc(1, 1)
                psa_cm.__exit__(None, None, None)
                if parts == "noout":
                    nc.sync.dma_start(
                        out_d.rearrange("(a r) o -> r a o", r=128),
                        wv[:, 0:4, :])

        if loop_n is None:
            body()
        else:
            with tc.For_i(0, loop_n, 1, hint_engines=(
                    mybir.EngineType.PE, mybir.EngineType.Activation,
                    mybir.EngineType.DVE, mybir.EngineType.SP,
                    mybir.EngineType.Pool)):
                body()
    nc.compile()
    return nc


def _get_nc(loop_n=None, parts="all"):
    key = ("nc", loop_n, parts)
    if key not in _cache:
        _cache[key] = _build(loop_n, parts)
    return _cache[key]


def _bf16(a):
    import ml_dtypes
    return np.ascontiguousarray(a.astype(ml_dtypes.bfloat16))


def pack_inputs(x, Wq, bq, Wk, bk, Wv, bv, Wo, bo):
    x = np.asarray(x, np.float32)
    WqT = _bf16(np.asarray(Wq, np.float32).T)
    WkT = _bf16(np.asarray(Wk, np.float32).T)
    WvT = _bf16(np.asarray(Wv, np.float32).T)
    # woTre[64g + d, m, o] = Wo[o, 512g + 64m + d]
    WoTre = _bf16(
        np.asarray(Wo, np.float32).T.reshape(2, 8, 64, E).transpose(0, 2, 1, 3)
        .reshape(128, 8, E))
    bqT = np.ascontiguousarray(np.asarray(bq, np.float32).reshape(8, 128).T)
    bkT8 = np.ascontiguousarray(
        (np.asarray(bk, np.float32) / 8.0).reshape(8, 128).T)
    bvrow = np.asarray(bv, np.float32).reshape(1, E)
    borow = np.asarray(bo, np.float32).reshape(1, E)

    in_maps = []
    for c in range(8):
        xTs = np.empty((E, R), np.float32)
        for p in range(2):
            h = 2 * c + p
            b_, mp_ = divmod(h, 8)
            xTs[:, RP * p:RP * (p + 1)] = x[b_, RP * mp_:RP * (mp_ + 1), :].T
        in_maps.append({
            "xT": _bf16(xTs), "wqT": WqT, "wkT": WkT,
            "wvT": WvT, "woTre": WoTre, "bqT": bqT, "bkT8": bkT8,
            "bvrow": bvrow, "borow": borow,
        })
    return in_maps


def kernel(x, Wq, bq, Wk, bk, Wv, bv, Wo, bo):
    in_maps = pack_inputs(x, Wq, bq, Wk, bk, Wv, bv, Wo, bo)
    nc = _get_nc()
    res = run_bass_kernel_spmd(nc, in_maps, core_ids=list(range(8)))
    out = np.empty((2, 2048, E), np.float32)
    for c in range(8):
        o = np.asarray(res.results[c]["out"], dtype=np.float32)
        for p in range(2):
            h = 2 * c + p
            b_, mp_ = divmod(h, 8)
            out[b_, RP * mp_:RP * (mp_ + 1), :] = o[RP * p:RP * (p + 1), :]
    return out
